# revision 7
# baseline (speedup 1.0000x reference)
"""Adaptive average pooling (512,512)->(7,7) over [16,512,512,64] f32.

Data-parallel over batch: 2 samples per NeuronCore across 8 cores.
Per core: stream x[b] as [H=512, W*C=32768] through SBUF in 2 MiB
per-h-block DMAs, casting f32->bf16 in-flight (SWDGE) so the TensorEngine
runs at full rate; stage 1 (H-window reduction) = TensorE matmul with an
exact 0/1 bf16 window mask as lhsT (PSUM f32-accumulated over 4 h-blocks
of 128); stage 2 (W-window reduction) = VectorE strided reduce_sum
straight out of PSUM into the [7, 7*64] output tile; one final f32 scale
by 1/(74*74). Memory-bound: 128 MiB HBM read per core. The last chunk is
split fine so little work trails the final input byte.
"""

import ml_dtypes
import numpy as np

import concourse.mybir as mybir
from concourse import bacc
from concourse.tile import TileContext
from concourse.bass_utils import run_bass_kernel_spmd

B, H, W, C = 16, 512, 512, 64
OH = OW = 7
N_CORES = 8
BPC = B // N_CORES          # samples per core
WC = W * C                  # 32768 f32 columns per h-row
P = 128                     # SBUF partitions
HB = H // P                 # 4 h-blocks
CH = 4096                   # wc columns per chunk (4 x 2 MiB f32 DMAs, one per h-block)
GRP = 2048                  # wc columns per PSUM accumulation group (32 w, 4 banks)
TAIL_CH = 1024              # fine-grained sub-chunks for the final chunk
F32 = mybir.dt.float32
BF16 = mybir.dt.bfloat16


def _pool_windows(in_size: int, out_size: int):
    """Same window math as the reference _pool_matrix."""
    o = np.arange(out_size, dtype=np.float32)
    start = (o * in_size / out_size).astype(np.int32)
    stop = np.ceil((o + 1) * in_size / out_size).astype(np.int32)
    return [(int(s), int(e)) for s, e in zip(start, stop)]


def _mask_weights() -> np.ndarray:
    """[H, OH] bf16 0/1 mask of the H pooling windows (exact in bf16)."""
    wt = np.zeros((H, OH), dtype=np.float32)
    for oh, (s, e) in enumerate(_pool_windows(H, OH)):
        wt[s:e, oh] = 1.0
    return wt.astype(ml_dtypes.bfloat16)


def _out_scale() -> float:
    h_len = _pool_windows(H, OH)[0][1] - _pool_windows(H, OH)[0][0]
    w_len = _pool_windows(W, OW)[0][1] - _pool_windows(W, OW)[0][0]
    return 1.0 / (h_len * w_len)


def _build():
    nc = bacc.Bacc(None, target_bir_lowering=False)
    x = nc.dram_tensor("x", [BPC, H, W, C], F32, kind="ExternalInput")
    wt = nc.dram_tensor("wt", [H, OH], BF16, kind="ExternalInput")
    out = nc.dram_tensor("out", [BPC, OH, OW, C], F32, kind="ExternalOutput")

    # [b, hb, p, wc]: partition = h within block; one DMA per (b, hb, chunk)
    xv = x[:].rearrange("b (hb p) w c -> b hb p (w c)", p=P)
    wv = wt[:].rearrange("(hb p) o -> p hb o", p=P)
    ov = out[:].rearrange("b oh ow c -> b oh (ow c)")

    w_windows = _pool_windows(W, OW)

    # per-sample chunk lists; the very last chunk is split fine to shorten
    # the post-stream tail
    def chunks_for(b):
        cols = [(j * CH, CH) for j in range(WC // CH)]
        if b == BPC - 1:
            base, width = cols.pop()
            cols += [(base + k * TAIL_CH, TAIL_CH) for k in range(width // TAIL_CH)]
        return cols

    with TileContext(nc) as tc:
        with (
            tc.tile_pool(name="const", bufs=1) as const,
            tc.tile_pool(name="xin", bufs=12) as xin,
            tc.tile_pool(name="psum", bufs=2, space="PSUM") as psum,
            tc.tile_pool(name="tmp", bufs=4) as tmp,
            tc.tile_pool(name="yout", bufs=BPC) as yout,
        ):
            wt_sb = const.tile([P, HB, OH], BF16)
            nc.sync.dma_start(out=wt_sb[:], in_=wv)

            y_tiles = []
            for b in range(BPC):
                y = yout.tile([OH, OW * C], F32, tag=f"y{b}")
                y_tiles.append(y)
                started = [False] * OW
                for col0, width in chunks_for(b):
                    xts = []
                    for hb in range(HB):
                        xt = xin.tile([P, width], BF16, tag="xt")
                        nc.gpsimd.dma_start(  # f32 -> bf16 cast in flight
                            out=xt[:], in_=xv[b, hb, :, col0 : col0 + width]
                        )
                        xts.append(xt)
                    for g0 in range(0, width, GRP):
                        gw = min(GRP, width - g0)
                        ps = psum.tile([OH, gw], F32, tag="ps")
                        for q in range(gw // 512):
                            col = g0 + q * 512
                            for hb in range(HB):
                                nc.tensor.matmul(
                                    ps[:, q * 512 : (q + 1) * 512],
                                    wt_sb[:, hb, :],
                                    xts[hb][:, col : col + 512],
                                    start=(hb == 0),
                                    stop=(hb == HB - 1),
                                )
                        # stage 2: this group covers w in [w0, w0+gw/C)
                        w0 = (col0 + g0) // C
                        for ow, (ws, we) in enumerate(w_windows):
                            s = max(ws, w0)
                            e = min(we, w0 + gw // C)
                            if s >= e:
                                continue
                            seg = ps[:, (s - w0) * C : (e - w0) * C].rearrange(
                                "p (w c) -> p c w", c=C
                            )
                            yslice = y[:, ow * C : (ow + 1) * C]
                            if not started[ow]:
                                nc.vector.reduce_sum(
                                    yslice, seg, axis=mybir.AxisListType.X
                                )
                                started[ow] = True
                            else:
                                t = tmp.tile([OH, C], F32)
                                nc.vector.reduce_sum(
                                    t[:], seg, axis=mybir.AxisListType.X
                                )
                                nc.vector.tensor_add(out=yslice, in0=yslice, in1=t[:])
                nc.scalar.mul(y[:], y[:], _out_scale())
            for b in range(BPC):
                nc.scalar.dma_start(out=ov[b], in_=y_tiles[b][:])
    nc.finalize()
    return nc


_NC_CACHE = []


def kernel(x: np.ndarray) -> np.ndarray:
    assert x.shape == (B, H, W, C), x.shape
    x = np.ascontiguousarray(x, dtype=np.float32)
    wt = _mask_weights()
    if not _NC_CACHE:
        _NC_CACHE.append(_build())
    nc = _NC_CACHE[0]
    in_maps = [
        {"x": x[i * BPC : (i + 1) * BPC], "wt": wt} for i in range(N_CORES)
    ]
    res = run_bass_kernel_spmd(nc, in_maps, core_ids=list(range(N_CORES)))
    return np.concatenate([res.results[i]["out"] for i in range(N_CORES)], axis=0)


# revision 11
# speedup vs baseline: 1.2588x; 1.2588x over previous
"""Adaptive average pooling (512,512)->(7,7) over [16,512,512,64] f32.

Data-parallel over batch: 2 samples per NeuronCore across 8 cores.
Per core: stream x[b] as [H=512, W*C=32768] through SBUF in 2 MiB
per-h-block DMAs, casting f32->bf16 in-flight (SWDGE) so the TensorEngine
runs at full rate; stage 1 (H-window reduction) = TensorE matmul with an
exact 0/1 bf16 window mask as lhsT (PSUM f32-accumulated over 4 h-blocks
of 128); stage 2 (W-window reduction) = VectorE strided reduce_sum
straight out of PSUM into the [7, 7*64] output tile; one final f32 scale
by 1/(74*74). Memory-bound: 128 MiB HBM read per core. The last chunk is
split fine so little work trails the final input byte.
"""

import ml_dtypes
import numpy as np

import concourse.mybir as mybir
from concourse import bacc
from concourse.tile import TileContext
from concourse.bass_utils import run_bass_kernel_spmd

B, H, W, C = 16, 512, 512, 64
OH = OW = 7
N_CORES = 8
BPC = B // N_CORES          # samples per core
WC = W * C                  # 32768 f32 columns per h-row
P = 128                     # SBUF partitions
HB = H // P                 # 4 h-blocks
CH = 4096                   # wc columns per chunk (4 x 2 MiB f32 DMAs, one per h-block)
GRP = 2048                  # wc columns per PSUM accumulation group (32 w, 4 banks)
TAIL_CH = 1024              # fine-grained sub-chunks for the final chunk
F32 = mybir.dt.float32
BF16 = mybir.dt.bfloat16


def _pool_windows(in_size: int, out_size: int):
    """Same window math as the reference _pool_matrix."""
    o = np.arange(out_size, dtype=np.float32)
    start = (o * in_size / out_size).astype(np.int32)
    stop = np.ceil((o + 1) * in_size / out_size).astype(np.int32)
    return [(int(s), int(e)) for s, e in zip(start, stop)]


def _mask_weights() -> np.ndarray:
    """[H, OH] bf16 0/1 mask of the H pooling windows (exact in bf16)."""
    wt = np.zeros((H, OH), dtype=np.float32)
    for oh, (s, e) in enumerate(_pool_windows(H, OH)):
        wt[s:e, oh] = 1.0
    return wt.astype(ml_dtypes.bfloat16)


def _out_scale() -> float:
    h_len = _pool_windows(H, OH)[0][1] - _pool_windows(H, OH)[0][0]
    w_len = _pool_windows(W, OW)[0][1] - _pool_windows(W, OW)[0][0]
    return 1.0 / (h_len * w_len)


def _build():
    nc = bacc.Bacc(None, target_bir_lowering=False)
    x = nc.dram_tensor("x", [BPC, H, W, C], F32, kind="ExternalInput")
    wt = nc.dram_tensor("wt", [H, OH], BF16, kind="ExternalInput")
    out = nc.dram_tensor("out", [BPC, OH, OW, C], F32, kind="ExternalOutput")

    # [b, hb, p, wc]: partition = h within block; one DMA per (b, hb, chunk)
    xv = x[:].rearrange("b (hb p) w c -> b hb p (w c)", p=P)
    wv = wt[:].rearrange("(hb p) o -> p hb o", p=P)
    ov = out[:].rearrange("b oh ow c -> b oh (ow c)")

    w_windows = _pool_windows(W, OW)

    # per-sample chunk lists; the very last chunk is split fine to shorten
    # the post-stream tail
    def chunks_for(b):
        cols = [(j * CH, CH) for j in range(WC // CH)]
        if b == BPC - 1:
            base, width = cols.pop()
            cols += [(base + k * TAIL_CH, TAIL_CH) for k in range(width // TAIL_CH)]
        return cols

    with TileContext(nc) as tc:
        with (
            tc.tile_pool(name="const", bufs=1) as const,
            tc.tile_pool(name="xin", bufs=16) as xin,
            tc.tile_pool(name="psum", bufs=2, space="PSUM") as psum,
            tc.tile_pool(name="tmp", bufs=4) as tmp,
            tc.tile_pool(name="yout", bufs=BPC) as yout,
        ):
            wt_sb = const.tile([P, HB, OH], BF16)
            nc.sync.dma_start(out=wt_sb[:], in_=wv)

            y_tiles = []
            for b in range(BPC):
                y = yout.tile([OH, OW * C], F32, tag=f"y{b}")
                y_tiles.append(y)
                started = [False] * OW
                for col0, width in chunks_for(b):
                    xts = []
                    for hb in range(HB):
                        xt = xin.tile([P, width], BF16, tag="xt")
                        nc.gpsimd.dma_start(  # f32 -> bf16 cast in flight
                            out=xt[:], in_=xv[b, hb, :, col0 : col0 + width]
                        )
                        xts.append(xt)
                    for g0 in range(0, width, GRP):
                        gw = min(GRP, width - g0)
                        ps = psum.tile([OH, gw], F32, tag="ps")
                        for q in range(gw // 512):
                            col = g0 + q * 512
                            for hb in range(HB):
                                nc.tensor.matmul(
                                    ps[:, q * 512 : (q + 1) * 512],
                                    wt_sb[:, hb, :],
                                    xts[hb][:, col : col + 512],
                                    start=(hb == 0),
                                    stop=(hb == HB - 1),
                                )
                        # stage 2: this group covers w in [w0, w0+gw/C)
                        w0 = (col0 + g0) // C
                        for ow, (ws, we) in enumerate(w_windows):
                            s = max(ws, w0)
                            e = min(we, w0 + gw // C)
                            if s >= e:
                                continue
                            seg = ps[:, (s - w0) * C : (e - w0) * C].rearrange(
                                "p (w c) -> p c w", c=C
                            )
                            yslice = y[:, ow * C : (ow + 1) * C]
                            if not started[ow]:
                                nc.vector.reduce_sum(
                                    yslice, seg, axis=mybir.AxisListType.X
                                )
                                started[ow] = True
                            else:
                                t = tmp.tile([OH, C], F32)
                                nc.vector.reduce_sum(
                                    t[:], seg, axis=mybir.AxisListType.X
                                )
                                nc.vector.tensor_add(out=yslice, in0=yslice, in1=t[:])
                nc.scalar.mul(y[:], y[:], _out_scale())
            for b in range(BPC):
                nc.scalar.dma_start(out=ov[b], in_=y_tiles[b][:])
    nc.finalize()
    return nc


_NC_CACHE = []


def kernel(x: np.ndarray) -> np.ndarray:
    assert x.shape == (B, H, W, C), x.shape
    x = np.ascontiguousarray(x, dtype=np.float32)
    wt = _mask_weights()
    if not _NC_CACHE:
        _NC_CACHE.append(_build())
    nc = _NC_CACHE[0]
    in_maps = [
        {"x": x[i * BPC : (i + 1) * BPC], "wt": wt} for i in range(N_CORES)
    ]
    res = run_bass_kernel_spmd(nc, in_maps, core_ids=list(range(N_CORES)))
    return np.concatenate([res.results[i]["out"] for i in range(N_CORES)], axis=0)
